# revision 34
# baseline (speedup 1.0000x reference)
"""DiffTransformerBlock on 8 Trainium2 NeuronCores (Bass/Tile).

Sharding: core c -> (batch b = c//4, head-group g = c%4; 4 heads each).
Activations kept transposed (feature, seq).  Attention-output partials
AllReduce'd within each 4-core batch group (residual folded as 0.25*x);
SwiGLU hidden-split with ReduceScatter over seq-quarters.

Rev A vs baseline:
- Per-head streaming: q/k/v weight slices are DMA'd and projected per
  head, so the Act-heavy exp phase of head h overlaps the PE-heavy
  projections of head h+1.
- Softmax row sums ride the E@V matmul as an appended ones-column in a
  transposed-U layout (out = [s, v|r]); ratio/subln then use cheap
  per-partition scalars (no gpsimd broadcasts, no PE rowsum matmuls).
- E@V and Wo run in fp8 e4m3 DoubleRow (2x PE throughput).  exp emits
  fp8 E directly (x0.5 via exp bias); V, un x16; Wo x2048 host-side.
  All scales are powers of two and folded into activation scales or
  absorbed by rmsnorm scale-invariance.
- U'' is transposed back to (feature, seq) via PE transposes.
- Wo/AllReduce/attn-norm/SwiGLU are pipelined over seq-halves: AR of
  half h overlaps FF of half h-1.  The attn-norm runs entirely on
  PE-queued ops (ones-matmul sum-of-squares, 1-partition broadcast
  matmul) so it never queues behind collectives on gpsimd.
Softmax normalization is folded algebraically:
U' = U1 - (lam*r1/r2)*U2 and subln absorbs 1/r1 plus all fp8 scales.
subln_w and (1 - lambda_init) are folded into Wo host-side.
attention_mask is all zeros by construction (spec fill=zeros); skipped.
"""

import sys
import contextlib

if '/opt/trn_rl_repo' not in sys.path:
    sys.path.insert(0, '/opt/trn_rl_repo')

import numpy as np
import ml_dtypes

import concourse.bass as bass  # noqa: F401
import concourse.bass_isa as bass_isa
import concourse.tile as tile
import concourse.mybir as mybir
from concourse import bacc
from concourse.bass_utils import run_bass_kernel_spmd

F32 = mybir.dt.float32
BF16 = mybir.dt.bfloat16
F8 = mybir.dt.float8e4
AF = mybir.ActivationFunctionType
ALU = mybir.AluOpType
PM = mybir.MatmulPerfMode

B, S, E = 2, 1024, 2048
H, D = 16, 128
HG = 4                      # heads per core
HF = HG * 2 * D             # 1024: per-core q/k/v feature slice
HID = 2 * E                 # 4096
HIDC = HID // 4             # 1024: per-core hidden slice
LAMBDA_INIT = 0.8
EPS = 1e-6
SCALE = 1.0 / float(np.sqrt(D))
N_CORES = 8
GROUPS = [[0, 1, 2, 3], [4, 5, 6, 7]]

EB = E // 128               # 16 e-blocks
KB = EB                     # contraction blocks over E
FB = HF // 128              # 8 q/k/v feature blocks
TB = S // 128               # 8 t-blocks
MB = HIDC // 128            # 8 hidden blocks
SQ = S // 4                 # 256: seq slice per core after ReduceScatter

SV = 16.0                   # fp8 scale for V
SUN = 16.0                  # fp8 scale for un
SWO = 2048.0                # fp8 scale for Wo (host side)
LN05 = float(np.log(0.5))   # exp bias => E scaled by 0.5


def _build_nc(reps=1):
    nc = bacc.Bacc("TRN2", target_bir_lowering=False, debug=False,
                   num_devices=N_CORES)

    xt_d = nc.dram_tensor("xt", [E, S], BF16, kind="ExternalInput")
    wq_d = nc.dram_tensor("wq", [E, HF], BF16, kind="ExternalInput")
    wk_d = nc.dram_tensor("wk", [E, HF], BF16, kind="ExternalInput")
    wv_d = nc.dram_tensor("wv", [E, HF], BF16, kind="ExternalInput")
    wo_d = nc.dram_tensor("wo", [HF, E], F8, kind="ExternalInput")
    w1_d = nc.dram_tensor("w1", [E, HIDC], BF16, kind="ExternalInput")
    w3_d = nc.dram_tensor("w3", [E, HIDC], BF16, kind="ExternalInput")
    w2_d = nc.dram_tensor("w2", [HIDC, E], BF16, kind="ExternalInput")
    normw_d = nc.dram_tensor("normw", [128, 2 * EB], F32,
                             kind="ExternalInput")
    lam_d = nc.dram_tensor("lam", [1, 1], F32, kind="ExternalInput")
    ident_d = nc.dram_tensor("ident", [128, 128], BF16,
                             kind="ExternalInput")
    out_d = nc.dram_tensor("out", [E, SQ], F32, kind="ExternalOutput")

    with tile.TileContext(nc) as tc, contextlib.ExitStack() as ctx:
        consts = ctx.enter_context(tc.tile_pool(name="consts", bufs=1))
        ones_f = consts.tile([128, 1], F32)
        nc.vector.memset(ones_f[:], 1.0)
        ones_b = consts.tile([128, 1], BF16)
        nc.vector.memset(ones_b[:], 1.0)
        ones_row = consts.tile([1, 128], BF16)
        nc.vector.memset(ones_row[:], 1.0)
        eps_t = consts.tile([128, 1], F32)
        nc.vector.memset(eps_t[:], EPS)
        ln05_t = consts.tile([128, 1], F32)
        nc.vector.memset(ln05_t[:], LN05)
        normw_sb = consts.tile([128, 2 * EB], F32)
        nc.sync.dma_start(out=normw_sb[:], in_=normw_d[:])
        lam_sb = consts.tile([1, 1], F32)
        nc.sync.dma_start(out=lam_sb[:], in_=lam_d[:])
        lam_bc = consts.tile([128, 1], F32)
        nc.gpsimd.partition_broadcast(lam_bc[:], lam_sb[:])
        ident = consts.tile([128, 128], BF16)
        nc.sync.dma_start(out=ident[:], in_=ident_d[:])

        dram = ctx.enter_context(
            tc.tile_pool(name="dram", bufs=1, space="DRAM"))
        # Shared-output collectives unsupported for 4-core groups; Local
        # internal DRAM outputs, bf16 payload.  AllReduce is chunked per
        # seq-half so norm+FF of half h overlap AR of h+1.
        SH = S // 2
        ar_ins = [dram.tile([E, SH], BF16, name=f"arin{i}")
                  for i in range(2)]
        ar_outs = [dram.tile([E, SH], BF16, name=f"arout{i}")
                   for i in range(2)]
        rs_ins = [dram.tile([4, E // 2, SQ], BF16, name=f"rsin{i}")
                  for i in range(2)]
        rs_outs = [dram.tile([E // 2, SQ], BF16, name=f"rsout{i}")
                   for i in range(2)]

        for _rep in range(reps):
            # ================= phase 1: attention =========================
            xtp = contextlib.ExitStack()
            xtpool = xtp.enter_context(tc.tile_pool(name="xtp", bufs=1))
            xt = xtpool.tile([128, EB, S], BF16)
            # chunked so the first projections start after 1/4 arrives
            for kc in range(4):
                nc.sync.dma_start(
                    out=xt[:, 4 * kc:4 * kc + 4, :],
                    in_=xt_d.rearrange("(k p) s -> p k s",
                                       p=128)[:, 4 * kc:4 * kc + 4, :])

            un_stack = contextlib.ExitStack()
            un_pool = un_stack.enter_context(tc.tile_pool(name="un", bufs=1))
            un = un_pool.tile([128, FB, S], F8)
            # prefetch Wo while the head loop computes
            wop = un_stack.enter_context(tc.tile_pool(name="wop", bufs=1))
            wo_sb = wop.tile([128, FB, E], F8)
            nc.sync.dma_start(
                out=wo_sb[:],
                in_=wo_d.rearrange("(k p) e -> p k e", p=128))

            with contextlib.ExitStack() as pa:
                whp = pa.enter_context(tc.tile_pool(name="whp", bufs=2))
                qkp = pa.enter_context(tc.tile_pool(name="qkp", bufs=2))
                vtp = pa.enter_context(tc.tile_pool(name="vtp", bufs=2))
                ep = pa.enter_context(tc.tile_pool(name="ep", bufs=2))
                upp = pa.enter_context(tc.tile_pool(name="upp", bufs=2))
                untp = pa.enter_context(tc.tile_pool(name="untp", bufs=3))
                rowp = pa.enter_context(tc.tile_pool(name="rowp", bufs=2))

                ps512 = pa.enter_context(
                    tc.tile_pool(name="ps512", bufs=2, space="PSUM"))
                ups = pa.enter_context(
                    tc.tile_pool(name="ups", bufs=1, space="PSUM"))
                tpps = pa.enter_context(
                    tc.tile_pool(name="tpps", bufs=2, space="PSUM"))

                for h in range(HG):
                    fsl = slice(2 * D * h, 2 * D * (h + 1))
                    wq_h = whp.tile([128, KB, 2 * D], BF16, tag="wq")
                    nc.sync.dma_start(
                        out=wq_h[:],
                        in_=wq_d.rearrange("(k p) f -> p k f", p=128)[:, :, fsl])
                    wk_h = whp.tile([128, KB, 2 * D], BF16, tag="wk")
                    nc.sync.dma_start(
                        out=wk_h[:],
                        in_=wk_d.rearrange("(k p) f -> p k f", p=128)[:, :, fsl])
                    wv_h = whp.tile([128, KB, 2 * D], BF16, tag="wv")
                    nc.sync.dma_start(
                        out=wv_h[:],
                        in_=wv_d.rearrange("(k p) f -> p k f", p=128)[:, :, fsl])

                    # ---- q/k projections: [d, s] layout
                    qt_h = qkp.tile([128, 2, S], BF16, tag="qt")
                    kt_h = qkp.tile([128, 2, S], BF16, tag="kt")
                    for wsb, dst in ((wq_h, qt_h), (wk_h, kt_h)):
                        for m2 in range(2):
                            for hf in range(2):
                                sl = slice(hf * 512, (hf + 1) * 512)
                                ps = ps512.tile([128, 512], F32, tag="pj")
                                for k in range(KB):
                                    nc.tensor.matmul(
                                        ps[:],
                                        wsb[:, k, m2 * 128:(m2 + 1) * 128],
                                        xt[:, k, sl],
                                        start=(k == 0), stop=(k == KB - 1))
                                nc.any.tensor_copy(out=dst[:, m2, sl],
                                                   in_=ps[:])

                    # ---- v projection: [t, v] layout + ones col, fp8
                    vt_h = vtp.tile([128, TB, 257], F8, tag="vt")
                    nc.vector.memset(vt_h[:, :, 256:257], 1.0)
                    for tb in range(TB):
                        ps = ps512.tile([128, 512], F32, tag="pj")
                        for k in range(KB):
                            nc.tensor.matmul(
                                ps[:, 0:256],
                                xt[:, k, tb * 128:(tb + 1) * 128],
                                wv_h[:, k, :],
                                start=(k == 0), stop=(k == KB - 1))
                        nc.scalar.activation(
                            out=vt_h[:, tb, 0:256], in_=ps[:, 0:256],
                            func=AF.Copy, scale=SV)

                    # ---- scores -> exp (fp8 E, x0.5 via bias)
                    es = []
                    for j in range(2):
                        e_t = ep.tile([128, TB, S], F8, tag=f"e{j}")
                        for tb in range(TB):
                            for hf in range(2):
                                sl = slice(hf * 512, (hf + 1) * 512)
                                ps = ps512.tile([128, 512], F32, tag="sc")
                                # separate tag ring from "pj": 2+2 banks
                                nc.tensor.matmul(
                                    ps[:],
                                    kt_h[:, j, tb * 128:(tb + 1) * 128],
                                    qt_h[:, j, sl],
                                    start=True, stop=True)
                                nc.scalar.activation(
                                    out=e_t[:, tb, sl], in_=ps[:],
                                    func=AF.Exp, scale=SCALE,
                                    bias=ln05_t[:, 0:1])
                        es.append(e_t)

                    # ---- U^T = [s, v|r] via fp8 DoubleRow; combine; subln
                    up_all = upp.tile([128, TB, 256], BF16, tag="up")
                    ss_all = rowp.tile([128, TB], F32, tag="ss")
                    for sb in range(TB):
                        ssl = slice(sb * 128, (sb + 1) * 128)
                        ut = []
                        for j in range(2):
                            pu = ups.tile([128, 257], F32, tag=f"u{j}")
                            for p in range(TB // 2):
                                nc.tensor.matmul(
                                    pu[:],
                                    es[j][:, 2 * p:2 * p + 2, ssl],
                                    vt_h[:, 2 * p:2 * p + 2, :],
                                    start=(p == 0), stop=(p == TB // 2 - 1),
                                    perf_mode=PM.DoubleRow)
                            ut.append(pu)
                        recip2 = rowp.tile([128, 1], F32, tag="rc")
                        nc.vector.reciprocal(out=recip2[:],
                                             in_=ut[1][:, 256:257])
                        # walrus: only one non-scalar PSUM input per
                        # TensorScalarPtr -> evac U2 to SBUF first
                        u2_sb = untp.tile([128, 256], BF16, tag="u2")
                        nc.any.tensor_copy(out=u2_sb[:], in_=ut[1][:, 0:256])
                        ratio = rowp.tile([128, 1], F32, tag="ratio")
                        nc.vector.scalar_tensor_tensor(
                            out=ratio[:], in0=recip2[:], scalar=lam_bc[:, 0:1],
                            in1=ut[0][:, 256:257], op0=ALU.mult, op1=ALU.mult)
                        nc.vector.scalar_tensor_tensor(
                            out=up_all[:, sb, :], in0=u2_sb[:],
                            scalar=ratio[:, 0:1], in1=ut[0][:, 0:256],
                            op0=ALU.mult, op1=ALU.add)
                        scrap = untp.tile([128, 256], BF16, tag="scrap")
                        nc.scalar.activation(
                            out=scrap[:], in_=up_all[:, sb, :],
                            func=AF.Square, accum_out=ss_all[:, sb:sb + 1])

                    lnr = rowp.tile([128, TB], F32, tag="lnr")
                    nc.scalar.activation(out=lnr[:], in_=ss_all[:],
                                         func=AF.Ln, scale=1.0 / 256.0,
                                         bias=eps_t[:, 0:1])
                    rstd = rowp.tile([128, TB], F32, tag="rstd")
                    nc.scalar.activation(out=rstd[:], in_=lnr[:],
                                         func=AF.Exp, scale=-0.5)

                    # ---- apply rstd; transpose back to [v, s]; un fp8
                    for sb in range(TB):
                        un_t = untp.tile([128, 256], BF16, tag="unt")
                        nc.vector.tensor_scalar(
                            out=un_t[:], in0=up_all[:, sb, :],
                            scalar1=rstd[:, sb:sb + 1], scalar2=None,
                            op0=ALU.mult)
                        for c in range(2):
                            tp = tpps.tile([128, 128], BF16, tag="tp")
                            nc.tensor.transpose(
                                tp[:], un_t[:, c * 128:(c + 1) * 128],
                                ident[:])
                            nc.scalar.activation(
                                out=un[:, 2 * h + c, sb * 128:(sb + 1) * 128],
                                in_=tp[:], func=AF.Copy, scale=SUN)

            # ==== Wo (fp8 DoubleRow) + residual, per seq-quarter -> AR(q) =
            with contextlib.ExitStack() as pw:
                wo_ps = pw.enter_context(
                    tc.tile_pool(name="wops", bufs=5, space="PSUM"))
                evac = pw.enter_context(tc.tile_pool(name="evac", bufs=4))
                for q in range(2):
                    qsl = slice(q * 512, (q + 1) * 512)
                    for m in range(EB):
                        ps = wo_ps.tile([128, 512], F32, tag="wops")
                        for kp in range(FB // 2):
                            nc.tensor.matmul(
                                ps[:],
                                wo_sb[:, 2 * kp:2 * kp + 2,
                                      m * 128:(m + 1) * 128],
                                un[:, 2 * kp:2 * kp + 2, qsl],
                                start=(kp == 0), stop=(kp == FB // 2 - 1),
                                perf_mode=PM.DoubleRow)
                        # psum = SUN*SWO*attn_emb_q; fold residual as
                        # (SUN*SWO/4)*x; rmsnorm absorbs the scale.
                        xqs = evac.tile([128, 512], BF16, tag="xqs")
                        nc.scalar.mul(out=xqs[:], in_=xt[:, m, qsl],
                                      mul=SUN * SWO / 4.0)
                        arow = evac.tile([128, 512], BF16, tag="arow")
                        nc.vector.tensor_add(out=arow[:], in0=ps[:],
                                             in1=xqs[:])
                        nc.sync.dma_start(
                            out=ar_ins[q][m * 128:(m + 1) * 128, :],
                            in_=arow[:])
                    nc.gpsimd.collective_compute(
                        "AllReduce", ALU.add, replica_groups=GROUPS,
                        ins=[ar_ins[q].opt()],
                        outs=[ar_outs[q].opt()])

            un_stack.close()
            xtp.close()

            # ====== attn-norm + SwiGLU, pipelined per seq-quarter =========
            pz = contextlib.ExitStack()
            xb_pool = pz.enter_context(tc.tile_pool(name="xb", bufs=1))
            xb = xb_pool.tile([128, EB, S], BF16)
            zp = pz.enter_context(tc.tile_pool(name="zp", bufs=1))
            zt = zp.tile([128, EB, S], BF16)
            swp = pz.enter_context(tc.tile_pool(name="swp", bufs=1))
            w1_sb = swp.tile([128, KB, HIDC], BF16)
            w3_sb = swp.tile([128, KB, HIDC], BF16)
            w2_sb = swp.tile([128, MB, E], BF16)
            nc.sync.dma_start(
                out=w1_sb[:], in_=w1_d.rearrange("(k p) h -> p k h", p=128))
            nc.sync.dma_start(
                out=w3_sb[:], in_=w3_d.rearrange("(k p) h -> p k h", p=128))
            nc.sync.dma_start(
                out=w2_sb[:], in_=w2_d.rearrange("(k p) e -> p k e", p=128))
            gp = pz.enter_context(tc.tile_pool(name="gp", bufs=1))
            g_sb = gp.tile([128, MB, S], BF16)

            with contextlib.ExitStack() as pn:
                sqp2 = pn.enter_context(tc.tile_pool(name="sqp2", bufs=3))
                bcp2 = pn.enter_context(tc.tile_pool(name="bcp2", bufs=2))
                sgp = pn.enter_context(tc.tile_pool(name="sgp", bufs=3))
                evac2 = pn.enter_context(tc.tile_pool(name="evac2", bufs=3))
                norm_ps = pn.enter_context(
                    tc.tile_pool(name="normps", bufs=1, space="PSUM"))
                h1p = pn.enter_context(
                    tc.tile_pool(name="h1p", bufs=1, space="PSUM"))
                h3p = pn.enter_context(
                    tc.tile_pool(name="h3p", bufs=1, space="PSUM"))
                ff_ps = pn.enter_context(
                    tc.tile_pool(name="ffps", bufs=2, space="PSUM"))

                for q in range(2):
                    qsl = slice(q * 512, (q + 1) * 512)
                    # ---- attn-norm of half q (PE-queued ss+broadcast:
                    # no gpsimd, so it pipelines behind AR(q) naturally)
                    ssps = norm_ps.tile([1, 512], F32, tag="ss")
                    for m in range(EB):
                        nc.sync.dma_start(
                            out=zt[:, m, qsl],
                            in_=ar_outs[q][m * 128:(m + 1) * 128, :])
                        sq = sqp2.tile([128, 512], BF16, tag="sq2")
                        nc.vector.tensor_mul(out=sq[:], in0=zt[:, m, qsl],
                                             in1=zt[:, m, qsl])
                        nc.tensor.matmul(ssps[:], ones_b[:], sq[:],
                                         start=(m == 0), stop=(m == EB - 1))
                    lnr2 = bcp2.tile([1, 512], F32, tag="lnr")
                    nc.scalar.activation(
                        out=lnr2[:], in_=ssps[:], func=AF.Ln,
                        scale=1.0 / (E * (SUN * SWO) ** 2),
                        bias=eps_t[0:1, :])
                    rstd2 = bcp2.tile([1, 512], BF16, tag="rstd")
                    nc.scalar.activation(out=rstd2[:], in_=lnr2[:],
                                         func=AF.Exp, scale=-0.5)
                    bcps = norm_ps.tile([128, 512], F32, tag="bc")
                    nc.tensor.matmul(bcps[:], ones_row[:], rstd2[:],
                                     start=True, stop=True)
                    for m in range(EB):
                        tmp = sqp2.tile([128, 512], BF16, tag="tmp2")
                        nc.vector.tensor_mul(out=tmp[:], in0=zt[:, m, qsl],
                                             in1=bcps[:])
                        # normw attn slot pre-scaled by 1/(SUN*SWO) host
                        nc.scalar.activation(out=xb[:, m, qsl], in_=tmp[:],
                                             func=AF.Copy,
                                             scale=normw_sb[:, m:m + 1])

                    # ---- SwiGLU h1/h3 of half q
                    for mh in range(MB):
                        p1t = h1p.tile([128, 512], F32, tag="h1")
                        p3t = h3p.tile([128, 512], F32, tag="h3")
                        for k in range(KB):
                            nc.tensor.matmul(
                                p1t[:],
                                w1_sb[:, k, mh * 128:(mh + 1) * 128],
                                xb[:, k, qsl],
                                start=(k == 0), stop=(k == KB - 1))
                            nc.tensor.matmul(
                                p3t[:],
                                w3_sb[:, k, mh * 128:(mh + 1) * 128],
                                xb[:, k, qsl],
                                start=(k == 0), stop=(k == KB - 1))
                        sg = sgp.tile([128, 512], BF16, tag="sg")
                        nc.scalar.activation(out=sg[:], in_=p1t[:],
                                             func=AF.Sigmoid)
                        ta = sgp.tile([128, 512], BF16, tag="ta")
                        nc.vector.tensor_mul(out=ta[:], in0=p1t[:],
                                             in1=sg[:])
                        nc.vector.tensor_mul(out=g_sb[:, mh, qsl],
                                             in0=p3t[:], in1=ta[:])

                    # ---- w2 of half q -> rs chunk rows
                    for m in range(EB):
                        ps = ff_ps.tile([128, 512], F32, tag="ffps")
                        for k in range(MB):
                            nc.tensor.matmul(
                                ps[:],
                                w2_sb[:, k, m * 128:(m + 1) * 128],
                                g_sb[:, k, qsl],
                                start=(k == 0), stop=(k == MB - 1))
                        xbq = evac2.tile([128, 512], BF16, tag="xbq")
                        nc.scalar.mul(out=xbq[:], in_=xb[:, m, qsl],
                                      mul=0.25)
                        frow = evac2.tile([128, 512], BF16, tag="frow")
                        nc.vector.tensor_add(out=frow[:], in0=ps[:],
                                             in1=xbq[:])
                        mi = m % 8
                        nc.sync.dma_start(
                            out=rs_ins[m // 8][2 * q:2 * q + 2,
                                               mi * 128:(mi + 1) * 128, :]
                                .rearrange("two p s -> p two s"),
                            in_=frow[:].rearrange("p (two s) -> p two s",
                                                  two=2))

                for i in range(2):
                    nc.gpsimd.collective_compute(
                        "ReduceScatter", ALU.add, replica_groups=GROUPS,
                        ins=[rs_ins[i].opt()],
                        outs=[rs_outs[i].opt()])

            pz.close()

            # ================= final norm -> out ==========================
            with contextlib.ExitStack() as pf:
                z2p = pf.enter_context(tc.tile_pool(name="z2p", bufs=1))
                z2 = z2p.tile([128, EB, SQ], BF16)
                sqp3 = pf.enter_context(tc.tile_pool(name="sqp3", bufs=3))
                rows3 = pf.enter_context(tc.tile_pool(name="rows3", bufs=2))
                bcp3 = pf.enter_context(tc.tile_pool(name="bcp3", bufs=1))
                outp = pf.enter_context(tc.tile_pool(name="outp", bufs=3))

                acc3 = bcp3.tile([128, SQ], F32)
                for m in range(EB):
                    mi = m % 8
                    nc.sync.dma_start(
                        out=z2[:, m, :],
                        in_=rs_outs[m // 8][mi * 128:(mi + 1) * 128, :])
                    if m == 0:
                        nc.vector.tensor_mul(out=acc3[:], in0=z2[:, m, :],
                                             in1=z2[:, m, :])
                    else:
                        sq = sqp3.tile([128, SQ], F32, tag="sq3")
                        nc.vector.tensor_mul(out=sq[:], in0=z2[:, m, :],
                                             in1=z2[:, m, :])
                        nc.vector.tensor_add(out=acc3[:], in0=acc3[:],
                                             in1=sq[:])
                ssb3 = bcp3.tile([128, SQ], F32, name="ssb3")
                nc.gpsimd.partition_all_reduce(
                    ssb3[:], acc3[:], channels=128,
                    reduce_op=bass_isa.ReduceOp.add)
                lnr3 = bcp3.tile([128, SQ], F32, name="lnr3")
                nc.scalar.activation(out=lnr3[:], in_=ssb3[:], func=AF.Ln,
                                     scale=1.0 / E, bias=eps_t[:, 0:1])
                bc3 = bcp3.tile([128, SQ], BF16)
                nc.scalar.activation(out=bc3[:], in_=lnr3[:],
                                     func=AF.Exp, scale=-0.5)
                for m in range(EB):
                    tmp = sqp3.tile([128, SQ], F32, tag="tmp3")
                    nc.vector.tensor_mul(out=tmp[:], in0=z2[:, m, :],
                                         in1=bc3[:])
                    ot = outp.tile([128, SQ], F32, tag="ot")
                    nc.scalar.activation(out=ot[:], in_=tmp[:], func=AF.Copy,
                                         scale=normw_sb[:, EB + m:EB + m + 1])
                    nc.sync.dma_start(out=out_d[m * 128:(m + 1) * 128, :],
                                      in_=ot[:])

    nc.finalize()
    return nc


_NC_CACHE = None


def _get_nc():
    global _NC_CACHE
    if _NC_CACHE is None:
        _NC_CACHE = _build_nc()
    return _NC_CACHE


def _bf(x):
    return np.ascontiguousarray(np.asarray(x, np.float32)).astype(
        ml_dtypes.bfloat16)


def _f8(x, scale):
    return np.ascontiguousarray(
        np.asarray(x, np.float32) * scale).astype(ml_dtypes.float8_e4m3)


def make_in_maps(input_embeddings, Wq, Wk, Wv, Wo,
                 lam_q1, lam_k1, lam_q2, lam_k2, subln_w,
                 attn_norm_w, ff_norm_w, w1, w3, w2):
    x = np.asarray(input_embeddings, np.float32)
    Wo = np.asarray(Wo, np.float32)
    subln_w = np.asarray(subln_w, np.float32)

    lam = (np.exp(np.dot(np.asarray(lam_q1, np.float64),
                         np.asarray(lam_k1, np.float64)))
           - np.exp(np.dot(np.asarray(lam_q2, np.float64),
                           np.asarray(lam_k2, np.float64)))
           + LAMBDA_INIT)
    # kernel uses ratio = lam_bc * r1/r2 then U' = U2*ratio + U1
    lam_arr = np.full((1, 1), -lam, np.float32)

    wo_scaled = Wo * (np.tile(subln_w, H) * (1.0 - LAMBDA_INIT))[:, None]

    normw = np.zeros((128, 2 * EB), np.float32)
    normw[:, :EB] = (np.asarray(attn_norm_w, np.float32) / (SUN * SWO)
                     ).reshape(EB, 128).T
    normw[:, EB:] = np.asarray(ff_norm_w, np.float32).reshape(EB, 128).T

    ident = np.eye(128, dtype=ml_dtypes.bfloat16)

    Wq = np.asarray(Wq, np.float32)
    Wk = np.asarray(Wk, np.float32)
    Wv = np.asarray(Wv, np.float32)
    w1 = np.asarray(w1, np.float32)
    w3 = np.asarray(w3, np.float32)
    w2 = np.asarray(w2, np.float32)

    xts = [_bf(x[b].T) for b in range(B)]
    in_maps = []
    for c in range(N_CORES):
        b, g = c // 4, c % 4
        sl = slice(HF * g, HF * (g + 1))
        in_maps.append({
            "xt": xts[b],
            "wq": _bf(Wq[:, sl]),
            "wk": _bf(Wk[:, sl]),
            "wv": _bf(Wv[:, sl]),
            "wo": _f8(wo_scaled[sl, :], SWO),
            "w1": _bf(w1[:, sl]),
            "w3": _bf(w3[:, sl]),
            "w2": _bf(w2[sl, :]),
            "normw": normw,
            "lam": lam_arr,
            "ident": ident,
        })
    return in_maps


def assemble(results):
    out = np.empty((B, S, E), np.float32)
    for c in range(N_CORES):
        b, g = c // 4, c % 4
        out[b, SQ * g:SQ * (g + 1), :] = np.asarray(results[c]["out"]).T
    return out


def kernel(input_embeddings, attention_mask, Wq, Wk, Wv, Wo,
           lam_q1, lam_k1, lam_q2, lam_k2, subln_w,
           attn_norm_w, ff_norm_w, w1, w3, w2):
    in_maps = make_in_maps(input_embeddings, Wq, Wk, Wv, Wo,
                           lam_q1, lam_k1, lam_q2, lam_k2, subln_w,
                           attn_norm_w, ff_norm_w, w1, w3, w2)
    nc = _get_nc()
    res = run_bass_kernel_spmd(nc, in_maps, core_ids=list(range(N_CORES)))
    return assemble(res.results)


# revision 39
# speedup vs baseline: 1.0233x; 1.0233x over previous
"""DiffTransformerBlock on 8 Trainium2 NeuronCores (Bass/Tile).

Sharding: core c -> (batch b = c//4, head-group g = c%4; 4 heads each).
Activations kept transposed (feature, seq).  Attention-output partials
AllReduce'd within each 4-core batch group (residual folded as 0.25*x);
SwiGLU hidden-split with ReduceScatter over seq-quarters.

Rev A vs baseline:
- Per-head streaming: q/k/v weight slices are DMA'd and projected per
  head, so the Act-heavy exp phase of head h overlaps the PE-heavy
  projections of head h+1.
- Softmax row sums ride the E@V matmul as an appended ones-column in a
  transposed-U layout (out = [s, v|r]); ratio/subln then use cheap
  per-partition scalars (no gpsimd broadcasts, no PE rowsum matmuls).
- E@V and Wo run in fp8 e4m3 DoubleRow (2x PE throughput).  exp emits
  fp8 E directly (x0.5 via exp bias); V, un x16; Wo x2048 host-side.
  All scales are powers of two and folded into activation scales or
  absorbed by rmsnorm scale-invariance.
- U'' is transposed back to (feature, seq) via PE transposes.
- Wo/AllReduce/attn-norm/SwiGLU are pipelined over seq-halves: AR of
  half h overlaps FF of half h-1.  The attn-norm runs entirely on
  PE-queued ops (ones-matmul sum-of-squares, 1-partition broadcast
  matmul) so it never queues behind collectives on gpsimd.
Softmax normalization is folded algebraically:
U' = U1 - (lam*r1/r2)*U2 and subln absorbs 1/r1 plus all fp8 scales.
subln_w and (1 - lambda_init) are folded into Wo host-side.
attention_mask is all zeros by construction (spec fill=zeros); skipped.
"""

import sys
import contextlib

if '/opt/trn_rl_repo' not in sys.path:
    sys.path.insert(0, '/opt/trn_rl_repo')

import numpy as np
import ml_dtypes

import concourse.bass as bass  # noqa: F401
import concourse.bass_isa as bass_isa
import concourse.tile as tile
import concourse.mybir as mybir
from concourse import bacc
from concourse.bass_utils import run_bass_kernel_spmd

F32 = mybir.dt.float32
BF16 = mybir.dt.bfloat16
F8 = mybir.dt.float8e4
AF = mybir.ActivationFunctionType
ALU = mybir.AluOpType
PM = mybir.MatmulPerfMode

B, S, E = 2, 1024, 2048
H, D = 16, 128
HG = 4                      # heads per core
HF = HG * 2 * D             # 1024: per-core q/k/v feature slice
HID = 2 * E                 # 4096
HIDC = HID // 4             # 1024: per-core hidden slice
LAMBDA_INIT = 0.8
EPS = 1e-6
SCALE = 1.0 / float(np.sqrt(D))
N_CORES = 8
GROUPS = [[0, 1, 2, 3], [4, 5, 6, 7]]

EB = E // 128               # 16 e-blocks
KB = EB                     # contraction blocks over E
FB = HF // 128              # 8 q/k/v feature blocks
TB = S // 128               # 8 t-blocks
MB = HIDC // 128            # 8 hidden blocks
SQ = S // 4                 # 256: seq slice per core after ReduceScatter

SV = 16.0                   # fp8 scale for V
SUN = 16.0                  # fp8 scale for un
SWO = 2048.0                # fp8 scale for Wo (host side)
LN05 = float(np.log(0.5))   # exp bias => E scaled by 0.5


def _build_nc(reps=1):
    nc = bacc.Bacc("TRN2", target_bir_lowering=False, debug=False,
                   num_devices=N_CORES)

    xt_d = nc.dram_tensor("xt", [E, S], BF16, kind="ExternalInput")
    wq_d = nc.dram_tensor("wq", [E, HF], BF16, kind="ExternalInput")
    wk_d = nc.dram_tensor("wk", [E, HF], BF16, kind="ExternalInput")
    wv_d = nc.dram_tensor("wv", [E, HF], BF16, kind="ExternalInput")
    wo_d = nc.dram_tensor("wo", [HF, E], F8, kind="ExternalInput")
    w1_d = nc.dram_tensor("w1", [E, HIDC], BF16, kind="ExternalInput")
    w3_d = nc.dram_tensor("w3", [E, HIDC], BF16, kind="ExternalInput")
    w2_d = nc.dram_tensor("w2", [HIDC, E], BF16, kind="ExternalInput")
    normw_d = nc.dram_tensor("normw", [128, 2 * EB], F32,
                             kind="ExternalInput")
    lam_d = nc.dram_tensor("lam", [1, 1], F32, kind="ExternalInput")
    ident_d = nc.dram_tensor("ident", [128, 128], BF16,
                             kind="ExternalInput")
    out_d = nc.dram_tensor("out", [E, SQ], F32, kind="ExternalOutput")

    with tile.TileContext(nc) as tc, contextlib.ExitStack() as ctx:
        consts = ctx.enter_context(tc.tile_pool(name="consts", bufs=1))
        ones_f = consts.tile([128, 1], F32)
        nc.vector.memset(ones_f[:], 1.0)
        ones_b = consts.tile([128, 1], BF16)
        nc.vector.memset(ones_b[:], 1.0)
        ones_row = consts.tile([1, 128], BF16)
        nc.vector.memset(ones_row[:], 1.0)
        eps_t = consts.tile([128, 1], F32)
        nc.vector.memset(eps_t[:], EPS)
        ln05_t = consts.tile([128, 1], F32)
        nc.vector.memset(ln05_t[:], LN05)
        normw_sb = consts.tile([128, 2 * EB], F32)
        nc.sync.dma_start(out=normw_sb[:], in_=normw_d[:])
        lam_sb = consts.tile([1, 1], F32)
        nc.sync.dma_start(out=lam_sb[:], in_=lam_d[:])
        lam_bc = consts.tile([128, 1], F32)
        nc.gpsimd.partition_broadcast(lam_bc[:], lam_sb[:])
        ident = consts.tile([128, 128], BF16)
        nc.sync.dma_start(out=ident[:], in_=ident_d[:])

        dram = ctx.enter_context(
            tc.tile_pool(name="dram", bufs=1, space="DRAM"))
        # Shared-output collectives unsupported for 4-core groups; Local
        # internal DRAM outputs, bf16 payload.  AllReduce is chunked per
        # seq-half so norm+FF of half h overlap AR of h+1.
        SH = S // 2
        ar_ins = [dram.tile([E, SH], BF16, name=f"arin{i}")
                  for i in range(2)]
        ar_outs = [dram.tile([E, SH], BF16, name=f"arout{i}")
                   for i in range(2)]
        rs_ins = [dram.tile([4, E // 2, SQ], BF16, name=f"rsin{i}")
                  for i in range(2)]
        rs_outs = [dram.tile([E // 2, SQ], BF16, name=f"rsout{i}")
                   for i in range(2)]

        for _rep in range(reps):
            # ================= phase 1: attention =========================
            xtp = contextlib.ExitStack()
            xtpool = xtp.enter_context(tc.tile_pool(name="xtp", bufs=1))
            xt = xtpool.tile([128, EB, S], BF16)
            # chunked so the first projections start after 1/4 arrives
            for kc in range(4):
                nc.sync.dma_start(
                    out=xt[:, 4 * kc:4 * kc + 4, :],
                    in_=xt_d.rearrange("(k p) s -> p k s",
                                       p=128)[:, 4 * kc:4 * kc + 4, :])

            un_stack = contextlib.ExitStack()
            un_pool = un_stack.enter_context(tc.tile_pool(name="un", bufs=1))
            un = un_pool.tile([128, FB, S], F8)
            # prefetch Wo while the head loop computes
            wop = un_stack.enter_context(tc.tile_pool(name="wop", bufs=1))
            wo_sb = wop.tile([128, FB, E], F8)
            nc.sync.dma_start(
                out=wo_sb[:],
                in_=wo_d.rearrange("(k p) e -> p k e", p=128))

            with contextlib.ExitStack() as pa:
                whp = pa.enter_context(tc.tile_pool(name="whp", bufs=2))
                qkp = pa.enter_context(tc.tile_pool(name="qkp", bufs=2))
                vtp = pa.enter_context(tc.tile_pool(name="vtp", bufs=2))
                ep = pa.enter_context(tc.tile_pool(name="ep", bufs=2))
                upp = pa.enter_context(tc.tile_pool(name="upp", bufs=2))
                untp = pa.enter_context(tc.tile_pool(name="untp", bufs=3))
                rowp = pa.enter_context(tc.tile_pool(name="rowp", bufs=2))

                ps512 = pa.enter_context(
                    tc.tile_pool(name="ps512", bufs=2, space="PSUM"))
                ups = pa.enter_context(
                    tc.tile_pool(name="ups", bufs=1, space="PSUM"))
                tpps = pa.enter_context(
                    tc.tile_pool(name="tpps", bufs=2, space="PSUM"))

                for h in range(HG):
                    fsl = slice(2 * D * h, 2 * D * (h + 1))
                    wq_h = whp.tile([128, KB, 2 * D], BF16, tag="wq")
                    nc.sync.dma_start(
                        out=wq_h[:],
                        in_=wq_d.rearrange("(k p) f -> p k f", p=128)[:, :, fsl])
                    wk_h = whp.tile([128, KB, 2 * D], BF16, tag="wk")
                    nc.sync.dma_start(
                        out=wk_h[:],
                        in_=wk_d.rearrange("(k p) f -> p k f", p=128)[:, :, fsl])
                    wv_h = whp.tile([128, KB, 2 * D], BF16, tag="wv")
                    nc.sync.dma_start(
                        out=wv_h[:],
                        in_=wv_d.rearrange("(k p) f -> p k f", p=128)[:, :, fsl])

                    # ---- q/k projections: [d, s] layout
                    qt_h = qkp.tile([128, 2, S], BF16, tag="qt")
                    kt_h = qkp.tile([128, 2, S], BF16, tag="kt")
                    for wsb, dst in ((wq_h, qt_h), (wk_h, kt_h)):
                        for m2 in range(2):
                            for hf in range(2):
                                sl = slice(hf * 512, (hf + 1) * 512)
                                ps = ps512.tile([128, 512], F32, tag="pj")
                                for k in range(KB):
                                    nc.tensor.matmul(
                                        ps[:],
                                        wsb[:, k, m2 * 128:(m2 + 1) * 128],
                                        xt[:, k, sl],
                                        start=(k == 0), stop=(k == KB - 1))
                                nc.any.tensor_copy(out=dst[:, m2, sl],
                                                   in_=ps[:])

                    # ---- v projection: [t, v] layout + ones col, fp8
                    vt_h = vtp.tile([128, TB, 257], F8, tag="vt")
                    nc.vector.memset(vt_h[:, :, 256:257], 1.0)
                    for tb in range(TB):
                        ps = ps512.tile([128, 512], F32, tag="pj")
                        for k in range(KB):
                            nc.tensor.matmul(
                                ps[:, 0:256],
                                xt[:, k, tb * 128:(tb + 1) * 128],
                                wv_h[:, k, :],
                                start=(k == 0), stop=(k == KB - 1))
                        nc.scalar.activation(
                            out=vt_h[:, tb, 0:256], in_=ps[:, 0:256],
                            func=AF.Copy, scale=SV)

                    # ---- scores -> exp (fp8 E, x0.5 via bias)
                    es = []
                    for j in range(2):
                        e_t = ep.tile([128, TB, S], F8, tag=f"e{j}")
                        for tb in range(TB):
                            for hf in range(2):
                                sl = slice(hf * 512, (hf + 1) * 512)
                                ps = ps512.tile([128, 512], F32, tag="sc")
                                # separate tag ring from "pj": 2+2 banks
                                nc.tensor.matmul(
                                    ps[:],
                                    kt_h[:, j, tb * 128:(tb + 1) * 128],
                                    qt_h[:, j, sl],
                                    start=True, stop=True)
                                nc.scalar.activation(
                                    out=e_t[:, tb, sl], in_=ps[:],
                                    func=AF.Exp, scale=SCALE,
                                    bias=ln05_t[:, 0:1])
                        es.append(e_t)

                    # ---- U^T = [s, v|r] via fp8 DoubleRow; combine; subln
                    up_all = upp.tile([128, TB, 256], BF16, tag="up")
                    ss_all = rowp.tile([128, TB], F32, tag="ss")
                    for sb in range(TB):
                        ssl = slice(sb * 128, (sb + 1) * 128)
                        ut = []
                        for j in range(2):
                            pu = ups.tile([128, 257], F32, tag=f"u{j}")
                            for p in range(TB // 2):
                                nc.tensor.matmul(
                                    pu[:],
                                    es[j][:, 2 * p:2 * p + 2, ssl],
                                    vt_h[:, 2 * p:2 * p + 2, :],
                                    start=(p == 0), stop=(p == TB // 2 - 1),
                                    perf_mode=PM.DoubleRow)
                            ut.append(pu)
                        recip2 = rowp.tile([128, 1], F32, tag="rc")
                        nc.vector.reciprocal(out=recip2[:],
                                             in_=ut[1][:, 256:257])
                        # walrus: only one non-scalar PSUM input per
                        # TensorScalarPtr -> evac U2 to SBUF first
                        u2_sb = untp.tile([128, 256], BF16, tag="u2")
                        nc.any.tensor_copy(out=u2_sb[:], in_=ut[1][:, 0:256])
                        ratio = rowp.tile([128, 1], F32, tag="ratio")
                        nc.vector.scalar_tensor_tensor(
                            out=ratio[:], in0=recip2[:], scalar=lam_bc[:, 0:1],
                            in1=ut[0][:, 256:257], op0=ALU.mult, op1=ALU.mult)
                        nc.vector.scalar_tensor_tensor(
                            out=up_all[:, sb, :], in0=u2_sb[:],
                            scalar=ratio[:, 0:1], in1=ut[0][:, 0:256],
                            op0=ALU.mult, op1=ALU.add)
                        scrap = untp.tile([128, 256], BF16, tag="scrap")
                        nc.scalar.activation(
                            out=scrap[:], in_=up_all[:, sb, :],
                            func=AF.Square, accum_out=ss_all[:, sb:sb + 1])

                    lnr = rowp.tile([128, TB], F32, tag="lnr")
                    nc.scalar.activation(out=lnr[:], in_=ss_all[:],
                                         func=AF.Ln, scale=1.0 / 256.0,
                                         bias=eps_t[:, 0:1])
                    rstd = rowp.tile([128, TB], F32, tag="rstd")
                    nc.scalar.activation(out=rstd[:], in_=lnr[:],
                                         func=AF.Exp, scale=-0.5)

                    # ---- apply rstd; transpose back to [v, s]; un fp8
                    for sb in range(TB):
                        un_t = untp.tile([128, 256], BF16, tag="unt")
                        nc.vector.tensor_scalar(
                            out=un_t[:], in0=up_all[:, sb, :],
                            scalar1=rstd[:, sb:sb + 1], scalar2=None,
                            op0=ALU.mult)
                        for c in range(2):
                            tp = tpps.tile([128, 128], BF16, tag="tp")
                            nc.tensor.transpose(
                                tp[:], un_t[:, c * 128:(c + 1) * 128],
                                ident[:])
                            nc.scalar.activation(
                                out=un[:, 2 * h + c, sb * 128:(sb + 1) * 128],
                                in_=tp[:], func=AF.Copy, scale=SUN)

            # ==== Wo (fp8 DoubleRow) + residual, per seq-quarter -> AR(q) =
            with contextlib.ExitStack() as pw:
                wo_ps = pw.enter_context(
                    tc.tile_pool(name="wops", bufs=5, space="PSUM"))
                evac = pw.enter_context(tc.tile_pool(name="evac", bufs=4))
                for q in range(2):
                    qsl = slice(q * 512, (q + 1) * 512)
                    for m in range(EB):
                        ps = wo_ps.tile([128, 512], F32, tag="wops")
                        for kp in range(FB // 2):
                            nc.tensor.matmul(
                                ps[:],
                                wo_sb[:, 2 * kp:2 * kp + 2,
                                      m * 128:(m + 1) * 128],
                                un[:, 2 * kp:2 * kp + 2, qsl],
                                start=(kp == 0), stop=(kp == FB // 2 - 1),
                                perf_mode=PM.DoubleRow)
                        # psum = SUN*SWO*attn_emb_q; fold residual as
                        # (SUN*SWO/4)*x; rmsnorm absorbs the scale.
                        xqs = evac.tile([128, 512], BF16, tag="xqs")
                        nc.scalar.mul(out=xqs[:], in_=xt[:, m, qsl],
                                      mul=SUN * SWO / 4.0)
                        arow = evac.tile([128, 512], BF16, tag="arow")
                        nc.vector.tensor_add(out=arow[:], in0=ps[:],
                                             in1=xqs[:])
                        nc.sync.dma_start(
                            out=ar_ins[q][m * 128:(m + 1) * 128, :],
                            in_=arow[:])
                    nc.gpsimd.collective_compute(
                        "AllReduce", ALU.add, replica_groups=GROUPS,
                        ins=[ar_ins[q].opt()],
                        outs=[ar_outs[q].opt()])

            un_stack.close()
            xtp.close()

            # ====== attn-norm + SwiGLU, pipelined per seq-quarter =========
            pz = contextlib.ExitStack()
            xb_pool = pz.enter_context(tc.tile_pool(name="xb", bufs=1))
            xb = xb_pool.tile([128, EB, S], BF16)
            zp = pz.enter_context(tc.tile_pool(name="zp", bufs=1))
            zt = zp.tile([128, EB, S], BF16)
            swp = pz.enter_context(tc.tile_pool(name="swp", bufs=1))
            w1_sb = swp.tile([128, KB, HIDC], BF16)
            w3_sb = swp.tile([128, KB, HIDC], BF16)
            w2_sb = swp.tile([128, MB, E], BF16)
            nc.sync.dma_start(
                out=w1_sb[:], in_=w1_d.rearrange("(k p) h -> p k h", p=128))
            nc.sync.dma_start(
                out=w3_sb[:], in_=w3_d.rearrange("(k p) h -> p k h", p=128))
            nc.sync.dma_start(
                out=w2_sb[:], in_=w2_d.rearrange("(k p) e -> p k e", p=128))
            gp = pz.enter_context(tc.tile_pool(name="gp", bufs=1))
            g_sb = gp.tile([128, MB, S], BF16)

            with contextlib.ExitStack() as pn:
                sqp2 = pn.enter_context(tc.tile_pool(name="sqp2", bufs=3))
                bcp2 = pn.enter_context(tc.tile_pool(name="bcp2", bufs=2))
                sgp = pn.enter_context(tc.tile_pool(name="sgp", bufs=3))
                evac2 = pn.enter_context(tc.tile_pool(name="evac2", bufs=3))
                norm_ps = pn.enter_context(
                    tc.tile_pool(name="normps", bufs=1, space="PSUM"))
                h1p = pn.enter_context(
                    tc.tile_pool(name="h1p", bufs=2, space="PSUM"))
                h3p = pn.enter_context(
                    tc.tile_pool(name="h3p", bufs=2, space="PSUM"))
                ff_ps = pn.enter_context(
                    tc.tile_pool(name="ffps", bufs=2, space="PSUM"))

                for q in range(2):
                    qsl = slice(q * 512, (q + 1) * 512)
                    # ---- attn-norm of half q (PE-queued ss+broadcast:
                    # no gpsimd, so it pipelines behind AR(q) naturally)
                    ssps = norm_ps.tile([1, 512], F32, tag="ss")
                    for m in range(EB):
                        nc.sync.dma_start(
                            out=zt[:, m, qsl],
                            in_=ar_outs[q][m * 128:(m + 1) * 128, :])
                        sq = sqp2.tile([128, 512], BF16, tag="sq2")
                        nc.vector.tensor_mul(out=sq[:], in0=zt[:, m, qsl],
                                             in1=zt[:, m, qsl])
                        nc.tensor.matmul(ssps[:], ones_b[:], sq[:],
                                         start=(m == 0), stop=(m == EB - 1))
                    lnr2 = bcp2.tile([1, 512], F32, tag="lnr")
                    nc.scalar.activation(
                        out=lnr2[:], in_=ssps[:], func=AF.Ln,
                        scale=1.0 / (E * (SUN * SWO) ** 2),
                        bias=eps_t[0:1, :])
                    rstd2 = bcp2.tile([1, 512], BF16, tag="rstd")
                    nc.scalar.activation(out=rstd2[:], in_=lnr2[:],
                                         func=AF.Exp, scale=-0.5)
                    bcps = norm_ps.tile([128, 512], F32, tag="bc")
                    nc.tensor.matmul(bcps[:], ones_row[:], rstd2[:],
                                     start=True, stop=True)
                    for m in range(EB):
                        # xb stays scaled by SUN*SWO; attn_norm_w/(SUN*SWO)
                        # is folded into w1/w3 rows host-side, so no
                        # per-block rescale pass is needed here.
                        nc.vector.tensor_mul(out=xb[:, m, qsl],
                                             in0=zt[:, m, qsl], in1=bcps[:])

                    # ---- SwiGLU h1/h3 of half q
                    for mh in range(MB):
                        p1t = h1p.tile([128, 512], F32, tag="h1")
                        p3t = h3p.tile([128, 512], F32, tag="h3")
                        for k in range(KB):
                            nc.tensor.matmul(
                                p1t[:],
                                w1_sb[:, k, mh * 128:(mh + 1) * 128],
                                xb[:, k, qsl],
                                start=(k == 0), stop=(k == KB - 1))
                            nc.tensor.matmul(
                                p3t[:],
                                w3_sb[:, k, mh * 128:(mh + 1) * 128],
                                xb[:, k, qsl],
                                start=(k == 0), stop=(k == KB - 1))
                        sg = sgp.tile([128, 512], BF16, tag="sg")
                        nc.scalar.activation(out=sg[:], in_=p1t[:],
                                             func=AF.Sigmoid)
                        ta = sgp.tile([128, 512], BF16, tag="ta")
                        nc.vector.tensor_mul(out=ta[:], in0=p1t[:],
                                             in1=sg[:])
                        nc.vector.tensor_mul(out=g_sb[:, mh, qsl],
                                             in0=p3t[:], in1=ta[:])

                    # ---- w2 of half q -> rs chunk rows
                    for m in range(EB):
                        ps = ff_ps.tile([128, 512], F32, tag="ffps")
                        for k in range(MB):
                            nc.tensor.matmul(
                                ps[:],
                                w2_sb[:, k, m * 128:(m + 1) * 128],
                                g_sb[:, k, qsl],
                                start=(k == 0), stop=(k == MB - 1))
                        xbq = evac2.tile([128, 512], BF16, tag="xbq")
                        # xb is SUN*SWO-scaled and carries no attn_norm_w;
                        # residual needs attn_norm_w applied -> only valid
                        # because the host folds it into w1/w3 AND scales
                        # xbq by attn_norm_w via normw_sb... but normw is
                        # per-partition; use Copy with scale AP instead.
                        nc.scalar.activation(
                            out=xbq[:], in_=xb[:, m, qsl], func=AF.Copy,
                            scale=normw_sb[:, m:m + 1])
                        frow = evac2.tile([128, 512], BF16, tag="frow")
                        nc.vector.tensor_add(out=frow[:], in0=ps[:],
                                             in1=xbq[:])
                        mi = m % 8
                        nc.sync.dma_start(
                            out=rs_ins[m // 8][2 * q:2 * q + 2,
                                               mi * 128:(mi + 1) * 128, :]
                                .rearrange("two p s -> p two s"),
                            in_=frow[:].rearrange("p (two s) -> p two s",
                                                  two=2))

                for i in range(2):
                    nc.gpsimd.collective_compute(
                        "ReduceScatter", ALU.add, replica_groups=GROUPS,
                        ins=[rs_ins[i].opt()],
                        outs=[rs_outs[i].opt()])

            pz.close()

            # ================= final norm -> out ==========================
            with contextlib.ExitStack() as pf:
                z2p = pf.enter_context(tc.tile_pool(name="z2p", bufs=1))
                z2 = z2p.tile([128, EB, SQ], BF16)
                sqp3 = pf.enter_context(tc.tile_pool(name="sqp3", bufs=3))
                rows3 = pf.enter_context(tc.tile_pool(name="rows3", bufs=2))
                bcp3 = pf.enter_context(tc.tile_pool(name="bcp3", bufs=1))
                outp = pf.enter_context(tc.tile_pool(name="outp", bufs=3))

                acc3 = bcp3.tile([128, SQ], F32)
                for m in range(EB):
                    mi = m % 8
                    nc.sync.dma_start(
                        out=z2[:, m, :],
                        in_=rs_outs[m // 8][mi * 128:(mi + 1) * 128, :])
                    if m == 0:
                        nc.vector.tensor_mul(out=acc3[:], in0=z2[:, m, :],
                                             in1=z2[:, m, :])
                    else:
                        sq = sqp3.tile([128, SQ], F32, tag="sq3")
                        nc.vector.tensor_mul(out=sq[:], in0=z2[:, m, :],
                                             in1=z2[:, m, :])
                        nc.vector.tensor_add(out=acc3[:], in0=acc3[:],
                                             in1=sq[:])
                ssb3 = bcp3.tile([128, SQ], F32, name="ssb3")
                nc.gpsimd.partition_all_reduce(
                    ssb3[:], acc3[:], channels=128,
                    reduce_op=bass_isa.ReduceOp.add)
                lnr3 = bcp3.tile([128, SQ], F32, name="lnr3")
                nc.scalar.activation(out=lnr3[:], in_=ssb3[:], func=AF.Ln,
                                     scale=1.0 / E, bias=eps_t[:, 0:1])
                bc3 = bcp3.tile([128, SQ], BF16)
                nc.scalar.activation(out=bc3[:], in_=lnr3[:],
                                     func=AF.Exp, scale=-0.5)
                for m in range(EB):
                    tmp = sqp3.tile([128, SQ], F32, tag="tmp3")
                    nc.vector.tensor_mul(out=tmp[:], in0=z2[:, m, :],
                                         in1=bc3[:])
                    ot = outp.tile([128, SQ], F32, tag="ot")
                    nc.scalar.activation(out=ot[:], in_=tmp[:], func=AF.Copy,
                                         scale=normw_sb[:, EB + m:EB + m + 1])
                    nc.sync.dma_start(out=out_d[m * 128:(m + 1) * 128, :],
                                      in_=ot[:])

    nc.finalize()
    return nc


_NC_CACHE = None


def _get_nc():
    global _NC_CACHE
    if _NC_CACHE is None:
        _NC_CACHE = _build_nc()
    return _NC_CACHE


def _bf(x):
    return np.ascontiguousarray(np.asarray(x, np.float32)).astype(
        ml_dtypes.bfloat16)


def _f8(x, scale):
    return np.ascontiguousarray(
        np.asarray(x, np.float32) * scale).astype(ml_dtypes.float8_e4m3)


def make_in_maps(input_embeddings, Wq, Wk, Wv, Wo,
                 lam_q1, lam_k1, lam_q2, lam_k2, subln_w,
                 attn_norm_w, ff_norm_w, w1, w3, w2):
    x = np.asarray(input_embeddings, np.float32)
    Wo = np.asarray(Wo, np.float32)
    subln_w = np.asarray(subln_w, np.float32)

    lam = (np.exp(np.dot(np.asarray(lam_q1, np.float64),
                         np.asarray(lam_k1, np.float64)))
           - np.exp(np.dot(np.asarray(lam_q2, np.float64),
                           np.asarray(lam_k2, np.float64)))
           + LAMBDA_INIT)
    # kernel uses ratio = lam_bc * r1/r2 then U' = U2*ratio + U1
    lam_arr = np.full((1, 1), -lam, np.float32)

    wo_scaled = Wo * (np.tile(subln_w, H) * (1.0 - LAMBDA_INIT))[:, None]

    anw = np.asarray(attn_norm_w, np.float32)
    normw = np.zeros((128, 2 * EB), np.float32)
    # attn slot: residual scale 0.25*attn_norm_w/(SUN*SWO) — xb tiles
    # stay SUN*SWO-scaled on device and attn_norm_w itself is folded
    # into the w1/w3 rows below.
    normw[:, :EB] = (0.25 * anw / (SUN * SWO)).reshape(EB, 128).T
    normw[:, EB:] = np.asarray(ff_norm_w, np.float32).reshape(EB, 128).T

    ident = np.eye(128, dtype=ml_dtypes.bfloat16)

    Wq = np.asarray(Wq, np.float32)
    Wk = np.asarray(Wk, np.float32)
    Wv = np.asarray(Wv, np.float32)
    wfold = (anw / (SUN * SWO))[:, None]
    w1 = np.asarray(w1, np.float32) * wfold
    w3 = np.asarray(w3, np.float32) * wfold
    w2 = np.asarray(w2, np.float32)

    xts = [_bf(x[b].T) for b in range(B)]
    in_maps = []
    for c in range(N_CORES):
        b, g = c // 4, c % 4
        sl = slice(HF * g, HF * (g + 1))
        in_maps.append({
            "xt": xts[b],
            "wq": _bf(Wq[:, sl]),
            "wk": _bf(Wk[:, sl]),
            "wv": _bf(Wv[:, sl]),
            "wo": _f8(wo_scaled[sl, :], SWO),
            "w1": _bf(w1[:, sl]),
            "w3": _bf(w3[:, sl]),
            "w2": _bf(w2[sl, :]),
            "normw": normw,
            "lam": lam_arr,
            "ident": ident,
        })
    return in_maps


def assemble(results):
    out = np.empty((B, S, E), np.float32)
    for c in range(N_CORES):
        b, g = c // 4, c % 4
        out[b, SQ * g:SQ * (g + 1), :] = np.asarray(results[c]["out"]).T
    return out


def kernel(input_embeddings, attention_mask, Wq, Wk, Wv, Wo,
           lam_q1, lam_k1, lam_q2, lam_k2, subln_w,
           attn_norm_w, ff_norm_w, w1, w3, w2):
    in_maps = make_in_maps(input_embeddings, Wq, Wk, Wv, Wo,
                           lam_q1, lam_k1, lam_q2, lam_k2, subln_w,
                           attn_norm_w, ff_norm_w, w1, w3, w2)
    nc = _get_nc()
    res = run_bass_kernel_spmd(nc, in_maps, core_ids=list(range(N_CORES)))
    return assemble(res.results)
